# revision 2
# baseline (speedup 1.0000x reference)
"""Bayesian linear layer (mean-field reparameterization) on 8 TRN2 NeuronCores.

out[b,o] = sum_i (eps_w[b,o,i]*exp(w_psi[o,i]) + w_mu[o,i]) * x[b,i]
         + eps_b[b,o]*exp(b_psi[o]) + b_mu[o]

v3 strategy (data-parallel over batch, 32 batches/core):
 - Host: transpose eps_w shard to [i, b, o] (i-major), x shard to [i, b],
   psi/mu to [i, o]. All layout-only (no arithmetic on host).
 - eps path: per (i-chunk, 8-batch group) SWDGE cast-DMA (4 MiB f32 read
   -> bf16 [128, 8, 1024] SBUF tile; 32 KiB contiguous per partition),
   one bf16 DVE multiply by sTb[ic] broadcast over the batch dim
   (2 elem/cyc), then 16 bf16 PE matvecs contracting i.
 - Matvec output: lhsT is a zero-padded [128, 32] bf16 matrix with x_b in
   column b, so row b of a single PSUM [32, 1024] accumulator receives
   the result directly (no staging/gather). The fp32 mu-term matmuls
   (x @ muT) accumulate into the SAME PSUM tile, so the epilogue is one
   DVE add of the bias rows + output DMA.
 - bias row: eps_b * exp(b_psi) + b_mu via partition_broadcast + DVE.
"""

import numpy as np

import os

BS, OUT, IN = 256, 1024, 1024
NCORES = 8
BPC = BS // NCORES          # 32 batches per core
ICH = IN // 128             # 8 i-chunks
EBUFS_DEFAULT = int(os.environ.get("BK_EBUFS", "6"))
PBUFS_DEFAULT = int(os.environ.get("BK_PBUFS", "2"))
MU_CPT = 4                  # i-chunks per mu/psi DMA (f32)

_cache = {}


def _build(reps, loop=False, ebufs=None, pbufs=None):
    EBUFS = ebufs or EBUFS_DEFAULT
    PBUFS = pbufs or PBUFS_DEFAULT
    import concourse.bass as bass
    import concourse.mybir as mybir
    import concourse.bacc as bacc
    from concourse import tile

    f32 = mybir.dt.float32
    bf16 = mybir.dt.bfloat16
    mult = mybir.AluOpType.mult
    add = mybir.AluOpType.add

    nc = bacc.Bacc(None, target_bir_lowering=False)

    d_eps = nc.dram_tensor("epsT", [IN, BPC, OUT], f32, kind="ExternalInput")
    d_xT = nc.dram_tensor("xT", [IN, BPC], f32, kind="ExternalInput")
    d_psiT = nc.dram_tensor("psiT", [IN, OUT], f32, kind="ExternalInput")
    d_muT = nc.dram_tensor("muT", [IN, OUT], f32, kind="ExternalInput")
    d_eb = nc.dram_tensor("eps_b", [BPC, OUT], f32, kind="ExternalInput")
    d_bpsi = nc.dram_tensor("bpsi", [1, OUT], f32, kind="ExternalInput")
    d_bmu = nc.dram_tensor("bmu", [1, OUT], f32, kind="ExternalInput")
    if loop:
        d_it = nc.dram_tensor("iters", [1, 1], mybir.dt.int32,
                              kind="ExternalInput")
    d_out = nc.dram_tensor("out", [BPC, OUT], f32, kind="ExternalOutput")

    with tile.TileContext(nc) as tc:
        with tc.tile_pool(name="const", bufs=1) as cpool, \
             tc.tile_pool(name="eps", bufs=EBUFS) as epool, \
             tc.tile_pool(name="p2b", bufs=PBUFS) as p2pool, \
             tc.tile_pool(name="ps", bufs=2, space="PSUM") as pspool:

            sTb = cpool.tile([128, ICH, OUT], bf16, name="sTb")
            xTf = cpool.tile([128, ICH, BPC], f32, name="xTf")
            xdiag = cpool.tile([128, ICH, BPC, BPC], bf16, name="xdiag")
            ebt = cpool.tile([BPC, OUT], f32, name="ebt")
            sbrow = cpool.tile([1, OUT], f32, name="sbrow")
            sb_bc = cpool.tile([BPC, OUT], f32, name="sb_bc")
            mu_bc = cpool.tile([BPC, OUT], f32, name="mu_bc")
            bias_rows = cpool.tile([BPC, OUT], f32, name="bias_rows")
            out_sb = cpool.tile([BPC, OUT], f32, name="out_sb")
            murow = cpool.tile([1, OUT], f32, name="murow")

            nc.vector.memset(xdiag[:], 0.0)

            def emit(rep):
                # ---- prologue: params, exp(psi) -> bf16, bias rows ----
                for t in range(ICH // MU_CPT):
                    pt = epool.tile([128, MU_CPT, OUT], f32,
                                    name=f"psi_{rep}_{t}", tag="eps")
                    nc.sync.dma_start(
                        out=pt[:],
                        in_=d_psiT[t * MU_CPT * 128:(t + 1) * MU_CPT * 128, :]
                        .rearrange("(s p) o -> p s o", p=128))
                    nc.scalar.activation(
                        sTb[:, t * MU_CPT:(t + 1) * MU_CPT, :], pt[:],
                        mybir.ActivationFunctionType.Exp)
                nc.sync.dma_start(out=xTf[:], in_=d_xT[:]
                                  .rearrange("(c p) b -> p c b", p=128))
                # xdiag[:, :, b, b] = x[b, :] ; off-diagonal stays zero
                for b in range(BPC):
                    nc.scalar.copy(xdiag[:, :, b, b], xTf[:, :, b])

                nc.sync.dma_start(out=ebt[:], in_=d_eb[:])
                nc.sync.dma_start(out=sbrow[:], in_=d_bpsi[:])
                nc.scalar.activation(sbrow[:], sbrow[:],
                                     mybir.ActivationFunctionType.Exp)
                nc.gpsimd.partition_broadcast(sb_bc[:], sbrow[:])
                nc.sync.dma_start(out=murow[:], in_=d_bmu[:])
                nc.gpsimd.partition_broadcast(mu_bc[:], murow[:])
                nc.vector.tensor_tensor(bias_rows[:], ebt[:], sb_bc[:], mult)
                nc.vector.tensor_tensor(bias_rows[:], bias_rows[:], mu_bc[:],
                                        add)

                # shared PSUM accumulator: mu-term + eps-term, rows = batches
                accps = pspool.tile([BPC, OUT], f32, name=f"acc_{rep}",
                                    tag="ps")

                # mu-term: acc[b, o] += sum_i x[b, i] * mu[o, i], M=32 fp32
                for t in range(ICH // MU_CPT):
                    mt = epool.tile([128, MU_CPT, OUT], f32,
                                    name=f"mu_{rep}_{t}", tag="eps")
                    nc.sync.dma_start(
                        out=mt[:],
                        in_=d_muT[t * MU_CPT * 128:(t + 1) * MU_CPT * 128, :]
                        .rearrange("(s p) o -> p s o", p=128))
                    for s in range(MU_CPT):
                        ic = t * MU_CPT + s
                        for h in range(2):
                            nc.tensor.matmul(
                                accps[:, h * 512:(h + 1) * 512],
                                xTf[:, ic, :],
                                mt[:, s, h * 512:(h + 1) * 512],
                                start=(ic == 0), stop=False)

                # ---- main loop: eps-term matvecs into the same PSUM ----
                BG = 8                       # batches per eps tile
                for ic in range(ICH):
                    for g in range(BPC // BG):
                        e = epool.tile([128, BG, OUT], bf16,
                                       name=f"e_{rep}_{ic}_{g}", tag="eps")
                        nc.gpsimd.dma_start(
                            out=e[:],
                            in_=d_eps[ic * 128:(ic + 1) * 128,
                                      g * BG:(g + 1) * BG, :])
                        p2 = p2pool.tile([128, BG, OUT], bf16,
                                         name=f"p2_{rep}_{ic}_{g}", tag="p2")
                        nc.vector.tensor_tensor(
                            p2[:], e[:],
                            sTb[:, ic:ic + 1, :].broadcast_to(
                                (128, BG, OUT)), mult)
                        last = ic == ICH - 1 and g == BPC // BG - 1
                        for bj in range(BG):
                            b = g * BG + bj
                            for h in range(2):
                                nc.tensor.matmul(
                                    accps[:, h * 512:(h + 1) * 512],
                                    xdiag[:, ic, b, :],
                                    p2[:, bj, h * 512:(h + 1) * 512],
                                    start=False,
                                    stop=(last and bj == BG - 1))

                # ---- epilogue: add bias rows, store ----
                nc.vector.tensor_tensor(out_sb[:], accps[:], bias_rows[:],
                                        add)
                nc.sync.dma_start(out=d_out[:], in_=out_sb[:])

            if loop:
                it_sb = cpool.tile([1, 1], mybir.dt.int32, name="it_sb")
                nc.sync.dma_start(out=it_sb[:], in_=d_it[:])
                regs = []
                for et in mybir.ALL_ENGINES:
                    eng = nc.engines[et]
                    r = eng.alloc_register(f"iters_{et.name}")
                    eng.reg_load(r, it_sb[0:1, 0:1])
                    regs.append(r)
                iters_val = bass.make_scalar_value(
                    bass.RegisterHandles(regs), min_val=1, max_val=1 << 20)
                with tc.For_i(0, iters_val, 1,
                              hint_engines=(mybir.EngineType.PE,
                                            mybir.EngineType.DVE,
                                            mybir.EngineType.SP)):
                    emit(0)
            else:
                for rep in range(reps):
                    emit(rep)

    nc.compile()
    return nc


def _get_nc(reps, loop=False, ebufs=None, pbufs=None):
    key = (reps, loop, ebufs, pbufs)
    if key not in _cache:
        _cache[key] = _build(reps, loop, ebufs, pbufs)
    return _cache[key]


def _prepare_inmaps(x, weight_mu, weight_psi, bias_mu, bias_psi, eps_w, eps_b):
    x = np.asarray(x, dtype=np.float32)
    weight_mu = np.asarray(weight_mu, dtype=np.float32)
    weight_psi = np.asarray(weight_psi, dtype=np.float32)
    bias_mu = np.asarray(bias_mu, dtype=np.float32)
    bias_psi = np.asarray(bias_psi, dtype=np.float32)
    eps_w = np.asarray(eps_w, dtype=np.float32)
    eps_b = np.asarray(eps_b, dtype=np.float32)

    psiT = np.ascontiguousarray(weight_psi.T)
    muT = np.ascontiguousarray(weight_mu.T)
    bpsi = bias_psi.reshape(1, OUT)
    bmu = bias_mu.reshape(1, OUT)

    in_maps = []
    for c in range(NCORES):
        sl = slice(c * BPC, (c + 1) * BPC)
        in_maps.append({
            "epsT": np.ascontiguousarray(eps_w[sl].transpose(2, 0, 1)),
            "xT": np.ascontiguousarray(x[sl].T),
            "psiT": psiT,
            "muT": muT,
            "eps_b": np.ascontiguousarray(eps_b[sl]),
            "bpsi": bpsi,
            "bmu": bmu,
        })
    return in_maps


def _run(in_maps, reps=1, loop_iters=None, ebufs=None, pbufs=None, **kw):
    from concourse.bass_utils import run_bass_kernel_spmd
    nc = _get_nc(reps, loop=loop_iters is not None, ebufs=ebufs, pbufs=pbufs)
    if loop_iters is not None:
        it = np.array([[loop_iters]], dtype=np.int32)
        in_maps = [{**m, "iters": it} for m in in_maps]
    res = run_bass_kernel_spmd(nc, in_maps, core_ids=list(range(NCORES)))
    return np.concatenate([res.results[c]["out"] for c in range(NCORES)],
                          axis=0)


def kernel(x, weight_mu, weight_psi, bias_mu, bias_psi, eps_w, eps_b):
    in_maps = _prepare_inmaps(x, weight_mu, weight_psi, bias_mu, bias_psi,
                              eps_w, eps_b)
    try:
        return _run(in_maps)
    except Exception:
        _cache.clear()
        return _run(in_maps)
